# revision 34
# baseline (speedup 1.0000x reference)
"""Trainium2 Bass kernel for DigitConvolutionalModel.

Computes: out = relu(conv2d_valid(x.reshape(B,28,28), w3x3)).reshape(B,676) @ W + b

Strategy (pure data parallel over 8 NeuronCores, 8192 images/core), bf16:
  - Host: pack x per core partition-major [120, 16 blocks, 7, 512] bf16.
    Pass-column c holds 2-row chunk c (rows 2c,2c+1) at partitions 0..55
    and chunk c+7 (rows 2c+14,2c+15) at partitions 64..119.
  - x streams in per-block DMA pairs: lo partitions [0:56] on the sync
    HWDGE ring, hi partitions [64:120] on the gpsimd SWDGE queue (both
    engines are compute-free so descriptor-gen never blocks behind
    compute; zero gap partitions are never transferred). dma_start for
    block j+1 is emitted after emit_conv(j) so each conv's DMA-wait
    threshold covers only blocks <= j. All of x stays resident in SBUF
    (~115KB/partition). 20 warm-up matmuls open the HAM clock gate and
    bridge the PE to block 1's arrival so it never re-throttles.
  - Conv per 512-image block: 7 passes. Pass p = 4 quadrant-tiled
    matmuls (2x2 tile_position grid, tile_size 64x64): chunk p lo rows x
    {A-taps -> group p, B-taps -> group p-1} and chunk p+7 hi rows x
    {A-taps -> group p+7, B-taps -> group p+6}. Groups g=0..12 are 2
    output rows (52 feats); group g accumulates A (chunk g) + B (chunk
    g+1) in PSUM partitions 64*(g%2)..+52 of pair bank g//2.
  - ReLU per pair bank [116,512] -> SBUF bf16 (gap partitions 52..63
    memset to 0 once per bank; FC weight rows there are 0).
  - FC: 7 matmuls (Wp chunks [116,32], col-tiled 4 strips) -> 2 rounds
    into one PSUM bank; DVE copy -> SBUF; Sel matmul [128,10] reduces
    the 4 col-strip partials; DVE bias-add. PSUM: pairs 0-5 on 6 stable
    psc banks; pair 6 ping-pongs with the FC partial bank (pso bufs=2)
    so the FC of block j+1 overlaps block j's Sel/bias drain.
  - FC for block j emitted after conv of block j+1 (software pipelining);
    all 16 outT stores are emitted after the block loop on the sync
    sequencer so their descriptor-gen (gated on each bias) never
    head-of-line-blocks the relu stream or the x DMA-lane thresholds.
"""

import numpy as np
import ml_dtypes

import concourse.bass as bass
import concourse.mybir as mybir
import concourse.tile as tile
from concourse import bacc
from concourse.bass_utils import run_bass_kernel_spmd

BF16 = ml_dtypes.bfloat16
PRECISION = "bf16"

# Problem geometry (fixed by the task spec)
B_FULL = 65536
IMG = 28
KW = 3
OH = IMG - KW + 1          # 26
NPIX = IMG * IMG           # 784
NFEAT = OH * OH            # 676
NOUT = 10

N_CORES = 8
B_CORE = B_FULL // N_CORES  # 8192
NB = 512                    # images per block
N_BLOCKS = B_CORE // NB     # 16

N_PASS = 7                  # conv passes per block (2 chunks each)
N_CHUNK = 14                # 2-row input chunks
N_GRP = 13                  # 2-out-row groups of 52 feats
G2 = 2 * OH                 # 52
XPART = 120                 # partitions used by the packed x layout
FCM = 32                    # FC stationary columns (one 32-col strip)
N_FC = 7                    # FC chunks (6 pairs of groups + 1 single)
KFC = 116                   # FC contraction for a group pair (52+12gap+52)

# Const blob column offsets (bf16, [128, BLOB_COLS])
OFF_SA = 0                  # [120, 52]  A-taps (lo rows 0..55, hi 64..119)
OFF_SB = 52                 # [120, 52]  B-taps
OFF_WP = 104                # [116, 7, 32] FC weights
OFF_SEL = 104 + 7 * FCM     # [128, 10]  col-strip reduction selector
BLOB_COLS = OFF_SEL + NOUT

WARM_MMS = 20               # HAM warm-up matmuls: open the clock gate AND
                            # bridge until x-hi(0) lands (~11us) so the PE
                            # never idles long enough to re-throttle

# Per-block x DMAs, split at the partition gap: lo partitions [0:56] ride
# the sync HWDGE ring (even SDMA engines), hi partitions [64:120] ride the
# gpsimd SWDGE queue (odd engines). Skips the zero gap partitions 56..63
# (-6.7% bytes). dma_start for block j+1 is emitted AFTER emit_conv(j) so
# each conv's DMA-wait threshold covers only blocks <= j (the scheduler
# bundles all earlier-emitted DMAs on a lane into the wait threshold).


def build_conv_mats(conv_w: np.ndarray):
    """SA[pixel, feat]: taps of a group's own chunk (input rows 2g,2g+1).
    SB[pixel, feat]: taps of the next chunk (rows 2g+2,2g+3). feat =
    26*l + oj for out row 2g+l, col oj."""
    w = np.asarray(conv_w, np.float32)
    SA = np.zeros((56, G2), np.float32)
    SB = np.zeros((56, G2), np.float32)
    for l in range(2):
        for oj in range(OH):
            f = OH * l + oj
            for r in range(2):
                for dj in range(KW):
                    c = oj + dj
                    diA = r - l
                    if 0 <= diA < KW:
                        SA[r * IMG + c, f] = w[diA, dj]
                    diB = 2 + r - l
                    if 0 <= diB < KW:
                        SB[r * IMG + c, f] = w[diB, dj]
    return SA, SB


def build_selector() -> np.ndarray:
    """S[32j + o, o] = 1: sums the 4 col-strip FC partials."""
    S = np.zeros((128, NOUT), np.float32)
    for j in range(4):
        for o in range(NOUT):
            S[FCM * j + o, o] = 1.0
    return S


def build_program():
    f32 = mybir.dt.float32
    bf = mybir.dt.bfloat16

    nc = bacc.Bacc()
    # Partition-major across blocks: per-partition bytes for a k-block
    # super-chunk DMA are contiguous (k*7168B descriptors amortize the
    # ~190ns per-descriptor overhead that caps small-descriptor DMAs).
    xP = nc.declare_dram_parameter("xP", [XPART, N_BLOCKS, N_PASS, NB], bf,
                                   isOutput=False)
    blob_d = nc.declare_dram_parameter("blob", [128, BLOB_COLS], bf,
                                       isOutput=False)
    bias_d = nc.declare_dram_parameter("bias", [NOUT, 1], f32, isOutput=False)
    outT = nc.declare_dram_parameter("outT", [NOUT, N_BLOCKS, NB], f32,
                                     isOutput=True)

    Relu = mybir.ActivationFunctionType.Relu
    Ident = mybir.ActivationFunctionType.Identity

    # Chunk visitation order: lo chunk LO_ORDER[p] and hi chunk
    # HI_ORDER[p] in pass p. Chosen so pair completions spread as
    # {p2: k0, p4: k1+k5, p5: k3+k6, p6: k2+k4} - only TWO pairs finish
    # at the last pass, each relu'd on a different engine, so the FC is
    # gated by ~one relu after pass 6 instead of a serial trio.
    LO_ORDER = (0, 1, 2, 3, 4, 6, 5)
    HI_ORDER = (7, 8, 10, 11, 12, 13, 9)
    RELU_AFTER = {2: [0], 4: [1, 5], 5: [3, 6], 6: [2, 4]}
    RELU_ON_ACT = {0, 1, 3, 2}  # 4 relus + bias-add on ACT; k5, k6, k4
    # + fcsb cast on DVE (balanced ~3.1us/engine per block)

    with tile.TileContext(nc) as tc:
        with (
            tc.tile_pool(name="const", bufs=1) as const,
            tc.tile_pool(name="feat", bufs=14) as fpool,
            tc.tile_pool(name="fcsb", bufs=2) as fcpool,
            tc.tile_pool(name="osb", bufs=16) as opool,
            tc.tile_pool(name="psc", bufs=6, space="PSUM") as psc,
            tc.tile_pool(name="pso", bufs=2, space="PSUM") as pso,
        ):
            # Constants on the scalar ring (sync ring is reserved for the
            # x-lo stream): one blob DMA (~150KB) plus the tiny f32 bias.
            blob = const.tile([128, BLOB_COLS], bf)
            nc.scalar.dma_start(out=blob[:, :], in_=blob_d[:, :])
            bias_sb = const.tile([NOUT, 1], f32)
            nc.scalar.dma_start(out=bias_sb[:, :], in_=bias_d[:, :])

            def SA(rb):   # stationary A-taps at row base rb (0 or 64)
                return blob[rb:rb + 56, OFF_SA:OFF_SA + G2]

            def SB(rb):
                return blob[rb:rb + 56, OFF_SB:OFF_SB + G2]

            def WP(t, k):
                return blob[0:k, OFF_WP + FCM * t:OFF_WP + FCM * (t + 1)]

            sel = blob[:, OFF_SEL:OFF_SEL + NOUT]

            # PE warm-up during the first x super-chunk's transfer. Source
            # is DVE-memset (no DMA dependency) so warm-up starts right
            # after the engine preamble and opens the HAM clock gate.
            wsrc = const.tile([128, NB], bf)
            nc.vector.memset(wsrc[:, :], 0)
            warm_ps = psc.tile([128, NB], mybir.dt.float32, tag="convps")
            for _ in range(WARM_MMS):
                nc.tensor.matmul(
                    warm_ps[:, :], wsrc[:, :128], wsrc[:, :],
                    start=True, stop=True,
                )
            # Zero all 8 psum banks once: ReLU reads [0:116] across the
            # gap partitions 52..63 (FC weight rows there are 0, but
            # relu(NaN-garbage) would still poison the FC), and the Sel
            # matmul reads unwritten rows of the FC partial bank.
            for _ in range(6):
                t = psc.tile([128, NB], mybir.dt.float32, tag="convps",
                             name="gapz")
                nc.vector.memset(t[:, :], 0)
            for _ in range(2):
                t = pso.tile([128, NB], mybir.dt.float32, tag="outps",
                             name="gapzo")
                nc.vector.memset(t[:, :], 0)

            xts = {}

            def issue_x(j):
                if j >= N_BLOCKS or j in xts:
                    return
                xt = const.tile([XPART, N_PASS, NB], bf, name=f"xb{j}")
                nc.sync.dma_start(out=xt[0:56, :, :],
                                  in_=xP[0:56, j, :, :])
                nc.gpsimd.dma_start(out=xt[64:120, :, :],
                                    in_=xP[64:120, j, :, :])
                xts[j] = xt

            state = {}

            def emit_conv(j, mid_cb=None):
                xt = xts[j]
                pairs = {}
                written = {}

                def pt(g):
                    k = g // 2
                    if k not in pairs:
                        # pair 6 (group 12 alone) ping-pongs with the FC
                        # partial bank in the pso pool: keeps the six psc
                        # banks on a stable pair->bank mapping.
                        pool = pso if k == 6 else psc
                        tag = "outps" if k == 6 else "convps"
                        pairs[k] = pool.tile([128, NB], mybir.dt.float32,
                                             tag=tag, name=f"pair{k}")
                    s = 64 * (g % 2)
                    return pairs[k][s:s + G2, :]

                def mm(g, stat, rb):
                    first = g not in written
                    written[g] = True
                    nc.tensor.matmul(
                        pt(g), stat(rb), xt[rb:rb + 56, p, :],
                        start=first, stop=not first,
                        tile_position=(rb, 64 * (g % 2)),
                        skip_group_check=True,
                    )

                feats = {}
                for p in range(N_PASS):
                    cl, ch = LO_ORDER[p], HI_ORDER[p]
                    mm(cl, SA, 0)                   # A: chunk cl -> group cl
                    if cl >= 1:
                        mm(cl - 1, SB, 0)           # B: chunk cl -> group cl-1
                    if ch <= 12:
                        mm(ch, SA, 64)              # A: chunk ch -> group ch
                    mm(ch - 1, SB, 64)              # B: chunk ch -> group ch-1
                    for k in RELU_AFTER.get(p, ()):
                        kf = KFC if k < N_FC - 1 else G2
                        ft = fpool.tile([kf, NB], bf, tag="feat", name=f"ft{k}")
                        if k in RELU_ON_ACT:
                            nc.scalar.activation(
                                ft[:, :], pairs[k][:kf, :], Relu)
                        else:
                            nc.vector.tensor_scalar_max(
                                ft[:, :], pairs[k][:kf, :], 0.0)
                        feats[k] = ft
                    if p == 2 and mid_cb is not None:
                        # Previous block's FC emitted mid-conv: its CAST
                        # lands early in the DVE FIFO (before this block's
                        # late relus) so Sel never stalls the PE stream.
                        mid_cb()
                state[j] = feats

            def emit_fc(j):
                feats = state.pop(j)
                # 7 col-tiled matmuls into one PSUM bank: round 1 strips
                # 0..3 (each clears its strip), round 2 strips 0..2 accum.
                ops = pso.tile([128, NB], mybir.dt.float32, tag="outps")
                for t in range(N_FC):
                    kf = KFC if t < N_FC - 1 else G2
                    strip = FCM * (t % 4)
                    nc.tensor.matmul(
                        ops[strip:strip + FCM, :], WP(t, kf),
                        feats[t][:, :],
                        start=(t < 4), stop=(t >= 3),
                        tile_position=(0, strip), skip_group_check=True,
                    )
                fcsb = fcpool.tile([128, NB], bf, tag="fcsb")
                nc.vector.tensor_copy(fcsb[:, :], ops[:, :])
                # Sel output reuses the fc-partial bank (already copied out).
                nc.tensor.matmul(
                    ops[:NOUT, :], sel[:, :], fcsb[:, :], start=True,
                    stop=True, skip_group_check=True,
                )
                osb = opool.tile([NOUT, NB], f32, tag="osb")
                # Bias-add on ACT (per-partition bias AP) to balance the
                # psum-read load across ACT and DVE.
                nc.scalar.activation(
                    osb[:, :], ops[:NOUT, :], Ident, bias=bias_sb[:, :]
                )
                osbs[j] = osb

            osbs = {}
            issue_x(0)
            for j in range(N_BLOCKS):
                cb = (lambda jj=j: emit_fc(jj - 1)) if j >= 1 else None
                emit_conv(j, cb)
                issue_x(j + 1)
            emit_fc(N_BLOCKS - 1)
            # Output stores last, on the sync sequencer (its x descgens all
            # run early): each store's descriptor-gen waits only its own
            # bias result, so stores pace with compute and never head-of-
            # line-block the relu stream or pollute x DMA-lane thresholds.
            for j in range(N_BLOCKS):
                nc.sync.dma_start(out=outT[:, j, :], in_=osbs[j][:, :])

    nc.finalize()
    return nc


def prepare_inputs(x, conv_w, W, b):
    SA, SB = build_conv_mats(conv_w)

    blob = np.zeros((128, BLOB_COLS), np.float32)
    blob[0:56, OFF_SA:OFF_SA + G2] = SA
    blob[64:120, OFF_SA:OFF_SA + G2] = SA
    blob[0:56, OFF_SB:OFF_SB + G2] = SB
    blob[64:120, OFF_SB:OFF_SB + G2] = SB

    Wf = np.asarray(W, np.float32)
    for t in range(N_FC):
        c0 = OFF_WP + FCM * t
        blob[0:G2, c0:c0 + NOUT] = Wf[G2 * 2 * t:G2 * (2 * t + 1), :]
        if t < N_FC - 1:
            blob[64:64 + G2, c0:c0 + NOUT] = Wf[G2 * (2 * t + 1):
                                                G2 * (2 * t + 2), :]
    blob[:, OFF_SEL:OFF_SEL + NOUT] = build_selector()
    blob = blob.astype(BF16)

    bias = np.asarray(b, np.float32).reshape(NOUT, 1)

    # Pack x: [B, 784] -> per core [120, N_BLOCKS, 7, NB] bf16
    # (partition-major across blocks for contiguous super-chunk DMAs).
    xbf = np.asarray(x, np.float32).astype(BF16)
    # [core, block, b, row, col] view of the batch-major input
    xv = xbf.reshape(N_CORES, N_BLOCKS, NB, IMG, IMG)
    in_maps = []
    for core in range(N_CORES):
        xp = np.zeros((XPART, N_BLOCKS, N_PASS, NB), BF16)
        LO_ORDER = (0, 1, 2, 3, 4, 6, 5)
        HI_ORDER = (7, 8, 10, 11, 12, 13, 9)
        for c in range(N_PASS):
            for r in range(2):
                # lo: chunk LO_ORDER[c]; hi: chunk HI_ORDER[c]
                xp[r * IMG:(r + 1) * IMG, :, c, :] = (
                    xv[core, :, :, 2 * LO_ORDER[c] + r, :].transpose(2, 0, 1)
                )
                xp[64 + r * IMG:64 + (r + 1) * IMG, :, c, :] = (
                    xv[core, :, :, 2 * HI_ORDER[c] + r, :].transpose(2, 0, 1)
                )
        in_maps.append({"xP": xp, "blob": blob, "bias": bias})
    return in_maps


def run(x, conv_w, W, b, trace=False, **spmd_kwargs):
    in_maps = prepare_inputs(x, conv_w, W, b)
    nc = build_program()
    res = run_bass_kernel_spmd(
        nc, in_maps, list(range(N_CORES)), trace=trace, **spmd_kwargs
    )
    out = np.empty((B_FULL, NOUT), np.float32)
    for c in range(N_CORES):
        out[c * B_CORE:(c + 1) * B_CORE, :] = (
            res.results[c]["outT"].reshape(NOUT, B_CORE).T
        )
    return out, res


def kernel(x, conv_w, W, b):
    out, _ = run(x, conv_w, W, b, trace=False)
    return out


# revision 35
# speedup vs baseline: 1.4436x; 1.4436x over previous
"""Trainium2 Bass kernel for DigitConvolutionalModel.

Computes: out = relu(conv2d_valid(x.reshape(B,28,28), w3x3)).reshape(B,676) @ W + b

Strategy (pure data parallel over 8 NeuronCores, 8192 images/core), bf16:
  - Host: pack x per core partition-major [120, 16 blocks, 7, 512] bf16.
    Pass-column c holds 2-row chunk c (rows 2c,2c+1) at partitions 0..55
    and chunk c+7 (rows 2c+14,2c+15) at partitions 64..119.
  - x streams in per-block DMA pairs: lo partitions [0:56] on the sync
    HWDGE ring, hi partitions [64:120] on the gpsimd SWDGE queue (both
    engines are compute-free so descriptor-gen never blocks behind
    compute; zero gap partitions are never transferred). dma_start for
    block j+1 is emitted after emit_conv(j) so each conv's DMA-wait
    threshold covers only blocks <= j. All of x stays resident in SBUF
    (~115KB/partition). 20 warm-up matmuls open the HAM clock gate and
    bridge the PE to block 1's arrival so it never re-throttles.
  - Conv per 512-image block: 7 passes. Pass p = 4 quadrant-tiled
    matmuls (2x2 tile_position grid, tile_size 64x64): chunk p lo rows x
    {A-taps -> group p, B-taps -> group p-1} and chunk p+7 hi rows x
    {A-taps -> group p+7, B-taps -> group p+6}. Groups g=0..12 are 2
    output rows (52 feats); group g accumulates A (chunk g) + B (chunk
    g+1) in PSUM partitions 64*(g%2)..+52 of pair bank g//2.
  - ReLU per pair bank [116,512] -> SBUF bf16 (gap partitions 52..63
    memset to 0 once per bank; FC weight rows there are 0).
  - FC: 7 matmuls (Wp chunks [116,32], col-tiled 4 strips) -> 2 rounds
    into one PSUM bank; DVE copy -> SBUF; Sel matmul [128,10] reduces
    the 4 col-strip partials; DVE bias-add. PSUM: pairs 0-5 on 6 stable
    psc banks; pair 6 ping-pongs with the FC partial bank (pso bufs=2)
    so the FC of block j+1 overlaps block j's Sel/bias drain.
  - FC for block j emitted after conv of block j+1 (software pipelining);
    all 16 outT stores are emitted after the block loop on the sync
    sequencer so their descriptor-gen (gated on each bias) never
    head-of-line-blocks the relu stream or the x DMA-lane thresholds.
"""

import numpy as np
import ml_dtypes

import concourse.bass as bass
import concourse.mybir as mybir
import concourse.tile as tile
from concourse import bacc
from concourse.bass_utils import run_bass_kernel_spmd

BF16 = ml_dtypes.bfloat16
PRECISION = "bf16"

# Problem geometry (fixed by the task spec)
B_FULL = 65536
IMG = 28
KW = 3
OH = IMG - KW + 1          # 26
NPIX = IMG * IMG           # 784
NFEAT = OH * OH            # 676
NOUT = 10

N_CORES = 8
B_CORE = B_FULL // N_CORES  # 8192
NB = 512                    # images per block
N_BLOCKS = B_CORE // NB     # 16

N_PASS = 7                  # conv passes per block (2 chunks each)
N_CHUNK = 14                # 2-row input chunks
N_GRP = 13                  # 2-out-row groups of 52 feats
G2 = 2 * OH                 # 52
XPART = 120                 # partitions used by the packed x layout
FCM = 32                    # FC stationary columns (one 32-col strip)
N_FC = 7                    # FC chunks (6 pairs of groups + 1 single)
KFC = 116                   # FC contraction for a group pair (52+12gap+52)

# Const blob column offsets (bf16, [128, BLOB_COLS])
OFF_SA = 0                  # [120, 52]  A-taps (lo rows 0..55, hi 64..119)
OFF_SB = 52                 # [120, 52]  B-taps
OFF_WP = 104                # [116, 7, 32] FC weights
OFF_SEL = 104 + 7 * FCM     # [128, 10]  col-strip reduction selector
BLOB_COLS = OFF_SEL + NOUT

WARM_MMS = 20               # HAM warm-up matmuls: open the clock gate AND
                            # bridge until x-hi(0) lands (~11us) so the PE
                            # never idles long enough to re-throttle

# Per-block x DMAs, split at the partition gap: lo partitions [0:56] ride
# the sync HWDGE ring (even SDMA engines), hi partitions [64:120] ride the
# gpsimd SWDGE queue (odd engines). Skips the zero gap partitions 56..63
# (-6.7% bytes). dma_start for block j+1 is emitted AFTER emit_conv(j) so
# each conv's DMA-wait threshold covers only blocks <= j (the scheduler
# bundles all earlier-emitted DMAs on a lane into the wait threshold).


def build_conv_mats(conv_w: np.ndarray):
    """SA[pixel, feat]: taps of a group's own chunk (input rows 2g,2g+1).
    SB[pixel, feat]: taps of the next chunk (rows 2g+2,2g+3). feat =
    26*l + oj for out row 2g+l, col oj."""
    w = np.asarray(conv_w, np.float32)
    SA = np.zeros((56, G2), np.float32)
    SB = np.zeros((56, G2), np.float32)
    for l in range(2):
        for oj in range(OH):
            f = OH * l + oj
            for r in range(2):
                for dj in range(KW):
                    c = oj + dj
                    diA = r - l
                    if 0 <= diA < KW:
                        SA[r * IMG + c, f] = w[diA, dj]
                    diB = 2 + r - l
                    if 0 <= diB < KW:
                        SB[r * IMG + c, f] = w[diB, dj]
    return SA, SB


def build_selector() -> np.ndarray:
    """S[32j + o, o] = 1: sums the 4 col-strip FC partials."""
    S = np.zeros((128, NOUT), np.float32)
    for j in range(4):
        for o in range(NOUT):
            S[FCM * j + o, o] = 1.0
    return S


def build_program():
    f32 = mybir.dt.float32
    bf = mybir.dt.bfloat16

    nc = bacc.Bacc()
    # Partition-major across blocks: per-partition bytes for a k-block
    # super-chunk DMA are contiguous (k*7168B descriptors amortize the
    # ~190ns per-descriptor overhead that caps small-descriptor DMAs).
    xP = nc.declare_dram_parameter("xP", [XPART, N_BLOCKS, N_PASS, NB], bf,
                                   isOutput=False)
    blob_d = nc.declare_dram_parameter("blob", [128, BLOB_COLS], bf,
                                       isOutput=False)
    bias_d = nc.declare_dram_parameter("bias", [NOUT, 1], f32, isOutput=False)
    outT = nc.declare_dram_parameter("outT", [NOUT, N_BLOCKS, NB], f32,
                                     isOutput=True)

    Relu = mybir.ActivationFunctionType.Relu
    Ident = mybir.ActivationFunctionType.Identity

    # pair index of the relu emitted after each conv pass
    RELU_AFTER = {2: [0], 3: [4], 4: [1], 5: [5], 6: [2, 3, 6]}
    RELU_ON_ACT = {0, 1, 2, 3, 6}  # pairs relu'd on ACT; rest on DVE

    with tile.TileContext(nc) as tc:
        with (
            tc.tile_pool(name="const", bufs=1) as const,
            tc.tile_pool(name="feat", bufs=14) as fpool,
            tc.tile_pool(name="fcsb", bufs=2) as fcpool,
            tc.tile_pool(name="osb", bufs=16) as opool,
            tc.tile_pool(name="psc", bufs=6, space="PSUM") as psc,
            tc.tile_pool(name="pso", bufs=2, space="PSUM") as pso,
        ):
            # Constants on the scalar ring (sync ring is reserved for the
            # x-lo stream): one blob DMA (~150KB) plus the tiny f32 bias.
            blob = const.tile([128, BLOB_COLS], bf)
            nc.scalar.dma_start(out=blob[:, :], in_=blob_d[:, :])
            bias_sb = const.tile([NOUT, 1], f32)
            nc.scalar.dma_start(out=bias_sb[:, :], in_=bias_d[:, :])

            def SA(rb):   # stationary A-taps at row base rb (0 or 64)
                return blob[rb:rb + 56, OFF_SA:OFF_SA + G2]

            def SB(rb):
                return blob[rb:rb + 56, OFF_SB:OFF_SB + G2]

            def WP(t, k):
                return blob[0:k, OFF_WP + FCM * t:OFF_WP + FCM * (t + 1)]

            sel = blob[:, OFF_SEL:OFF_SEL + NOUT]

            # PE warm-up during the first x super-chunk's transfer. Source
            # is DVE-memset (no DMA dependency) so warm-up starts right
            # after the engine preamble and opens the HAM clock gate.
            wsrc = const.tile([128, NB], bf)
            nc.vector.memset(wsrc[:, :], 0)
            warm_ps = psc.tile([128, NB], mybir.dt.float32, tag="convps")
            for _ in range(WARM_MMS):
                nc.tensor.matmul(
                    warm_ps[:, :], wsrc[:, :128], wsrc[:, :],
                    start=True, stop=True,
                )
            # Zero all 8 psum banks once: ReLU reads [0:116] across the
            # gap partitions 52..63 (FC weight rows there are 0, but
            # relu(NaN-garbage) would still poison the FC), and the Sel
            # matmul reads unwritten rows of the FC partial bank.
            for _ in range(6):
                t = psc.tile([128, NB], mybir.dt.float32, tag="convps",
                             name="gapz")
                nc.vector.memset(t[:, :], 0)
            for _ in range(2):
                t = pso.tile([128, NB], mybir.dt.float32, tag="outps",
                             name="gapzo")
                nc.vector.memset(t[:, :], 0)

            xts = {}

            def issue_x(j):
                if j >= N_BLOCKS or j in xts:
                    return
                xt = const.tile([XPART, N_PASS, NB], bf, name=f"xb{j}")
                nc.sync.dma_start(out=xt[0:56, :, :],
                                  in_=xP[0:56, j, :, :])
                nc.gpsimd.dma_start(out=xt[64:120, :, :],
                                    in_=xP[64:120, j, :, :])
                xts[j] = xt

            state = {}

            def emit_conv(j):
                xt = xts[j]
                pairs = {}
                written = {}

                def pt(g):
                    k = g // 2
                    if k not in pairs:
                        # pair 6 (group 12 alone) ping-pongs with the FC
                        # partial bank in the pso pool: keeps the six psc
                        # banks on a stable pair->bank mapping.
                        pool = pso if k == 6 else psc
                        tag = "outps" if k == 6 else "convps"
                        pairs[k] = pool.tile([128, NB], mybir.dt.float32,
                                             tag=tag, name=f"pair{k}")
                    s = 64 * (g % 2)
                    return pairs[k][s:s + G2, :]

                def mm(g, stat, rb):
                    first = g not in written
                    written[g] = True
                    nc.tensor.matmul(
                        pt(g), stat(rb), xt[rb:rb + 56, p, :],
                        start=first, stop=not first,
                        tile_position=(rb, 64 * (g % 2)),
                        skip_group_check=True,
                    )

                feats = {}
                for p in range(N_PASS):
                    mm(p, SA, 0)                    # A: chunk p -> group p
                    if p >= 1:
                        mm(p - 1, SB, 0)            # B: chunk p -> group p-1
                    if p <= 5:
                        mm(p + 7, SA, 64)           # A: chunk p+7 -> group p+7
                    mm(p + 6, SB, 64)               # B: chunk p+7 -> group p+6
                    for k in RELU_AFTER.get(p, ()):
                        kf = KFC if k < N_FC - 1 else G2
                        ft = fpool.tile([kf, NB], bf, tag="feat", name=f"ft{k}")
                        if k in RELU_ON_ACT:
                            nc.scalar.activation(
                                ft[:, :], pairs[k][:kf, :], Relu)
                        else:
                            nc.vector.tensor_scalar_max(
                                ft[:, :], pairs[k][:kf, :], 0.0)
                        feats[k] = ft
                state[j] = feats

            def emit_fc(j):
                feats = state.pop(j)
                # 7 col-tiled matmuls into one PSUM bank: round 1 strips
                # 0..3 (each clears its strip), round 2 strips 0..2 accum.
                ops = pso.tile([128, NB], mybir.dt.float32, tag="outps")
                for t in range(N_FC):
                    kf = KFC if t < N_FC - 1 else G2
                    strip = FCM * (t % 4)
                    nc.tensor.matmul(
                        ops[strip:strip + FCM, :], WP(t, kf),
                        feats[t][:, :],
                        start=(t < 4), stop=(t >= 3),
                        tile_position=(0, strip), skip_group_check=True,
                    )
                fcsb = fcpool.tile([128, NB], bf, tag="fcsb")
                nc.vector.tensor_copy(fcsb[:, :], ops[:, :])
                # Sel output reuses the fc-partial bank (already copied out).
                nc.tensor.matmul(
                    ops[:NOUT, :], sel[:, :], fcsb[:, :], start=True,
                    stop=True, skip_group_check=True,
                )
                osb = opool.tile([NOUT, NB], f32, tag="osb")
                nc.vector.tensor_scalar(
                    osb[:, :], ops[:NOUT, :], bias_sb[:, :], None,
                    op0=mybir.AluOpType.add,
                )
                osbs[j] = osb

            osbs = {}
            issue_x(0)
            for j in range(N_BLOCKS):
                emit_conv(j)
                issue_x(j + 1)
                if j >= 1:
                    emit_fc(j - 1)
            emit_fc(N_BLOCKS - 1)
            # Output stores last, on the sync sequencer (its x descgens all
            # run early): each store's descriptor-gen waits only its own
            # bias result, so stores pace with compute and never head-of-
            # line-block the relu stream or pollute x DMA-lane thresholds.
            for j in range(N_BLOCKS):
                nc.sync.dma_start(out=outT[:, j, :], in_=osbs[j][:, :])

    nc.finalize()
    return nc


def prepare_inputs(x, conv_w, W, b):
    SA, SB = build_conv_mats(conv_w)

    blob = np.zeros((128, BLOB_COLS), np.float32)
    blob[0:56, OFF_SA:OFF_SA + G2] = SA
    blob[64:120, OFF_SA:OFF_SA + G2] = SA
    blob[0:56, OFF_SB:OFF_SB + G2] = SB
    blob[64:120, OFF_SB:OFF_SB + G2] = SB

    Wf = np.asarray(W, np.float32)
    for t in range(N_FC):
        c0 = OFF_WP + FCM * t
        blob[0:G2, c0:c0 + NOUT] = Wf[G2 * 2 * t:G2 * (2 * t + 1), :]
        if t < N_FC - 1:
            blob[64:64 + G2, c0:c0 + NOUT] = Wf[G2 * (2 * t + 1):
                                                G2 * (2 * t + 2), :]
    blob[:, OFF_SEL:OFF_SEL + NOUT] = build_selector()
    blob = blob.astype(BF16)

    bias = np.asarray(b, np.float32).reshape(NOUT, 1)

    # Pack x: [B, 784] -> per core [120, N_BLOCKS, 7, NB] bf16
    # (partition-major across blocks for contiguous super-chunk DMAs).
    xbf = np.asarray(x, np.float32).astype(BF16)
    # [core, block, b, row, col] view of the batch-major input
    xv = xbf.reshape(N_CORES, N_BLOCKS, NB, IMG, IMG)
    in_maps = []
    for core in range(N_CORES):
        xp = np.zeros((XPART, N_BLOCKS, N_PASS, NB), BF16)
        for c in range(N_PASS):
            for r in range(2):
                # lo: chunk c rows 2c+r; hi: chunk c+7 rows 2c+14+r
                xp[r * IMG:(r + 1) * IMG, :, c, :] = (
                    xv[core, :, :, 2 * c + r, :].transpose(2, 0, 1)
                )
                xp[64 + r * IMG:64 + (r + 1) * IMG, :, c, :] = (
                    xv[core, :, :, 2 * c + 14 + r, :].transpose(2, 0, 1)
                )
        in_maps.append({"xP": xp, "blob": blob, "bias": bias})
    return in_maps


def run(x, conv_w, W, b, trace=False, **spmd_kwargs):
    in_maps = prepare_inputs(x, conv_w, W, b)
    nc = build_program()
    res = run_bass_kernel_spmd(
        nc, in_maps, list(range(N_CORES)), trace=trace, **spmd_kwargs
    )
    out = np.empty((B_FULL, NOUT), np.float32)
    for c in range(N_CORES):
        out[c * B_CORE:(c + 1) * B_CORE, :] = (
            res.results[c]["outT"].reshape(NOUT, B_CORE).T
        )
    return out, res


def kernel(x, conv_w, W, b):
    out, _ = run(x, conv_w, W, b, trace=False)
    return out
